# revision 15
# baseline (speedup 1.0000x reference)
"""AttenGcnLayer Trainium2 kernel (8 NeuronCores, SPMD).

Strategy: edges are bucketed on host by destination (tail) node; nodes are
range-partitioned across the 8 cores, so every edge is device-local to the
core owning its tail (no collectives). Per-node-group segment softmax and
weighted scatter-sum are computed with one-hot matmuls accumulated in PSUM.

Algebraic restructuring (all weight-folding done on host in fp64):
  mess      = tanh(xW[head] + rWb[attr]) with xW = x @ Wm1.T (device phase-0,
              bf16 table in HBM), rWb = BN(r) @ Wm2.T + b_mess (device, tiny)
  att logit c = sum_h' a*lrelu(mess @ W1.T + bias)
            = 0.99*(t1 + t2) + 0.01*cz
    t1 = (0.01/0.99) * mess . u2            (u2 = W1.T a; DVE row-reduce)
    t2 = sum_h' s_h' * relu(mess @ W1as.T + bias_as)  (sign-folded weights;
         relu is a single fused DVE tensor_scalar op; sign sum via +-1 mask)
  softmax over segments: exp without max-subtraction (logits ~ N(0,1)),
  normalization U/(Z+eps) from an extra ones-column in the scatter matmul.
  gating softmax over 2 = sigmoid of logit difference, same relu-fold.
"""

import sys
sys.path.insert(0, "/opt/trn_rl_repo")

import numpy as np
import ml_dtypes

HID = 128
N_NODES = 50000
N_REL = 500
N_EDGES = 625000
N_CORES = 8
BN_EPS = 1e-5
NODES_PER_CORE = N_NODES // N_CORES      # 6250
GROUP = 128
NG = (NODES_PER_CORE + GROUP - 1) // GROUP   # 49 groups/core
NODES_PAD = NG * GROUP                        # 6272
X_CHUNKS = (N_NODES + 127) // 128             # 391
X_PAD = X_CHUNKS * 128                        # 50048
LRELU = 0.01

bf16 = ml_dtypes.bfloat16
USE_CCE = True
import os
CUT = int(os.environ.get("KERNEL_CUT", "0"))
STAGE = int(os.environ.get("KERNEL_STAGE", "9"))
REPEAT = int(os.environ.get("KERNEL_REPEAT", "1"))  # 0=full, 1=phase1 only, 2=+phase0, 3=+edges-no-gating

_COMPILED = {}


def _host_prep(x, r, que_context, fin_que, edge_index, edge_attr,
               W_mess, b_mess, W_matt, b_matt, mess_atten_weight,
               W_xatt, b_xatt, x_atten_weight, bn_gamma, bn_beta):
    """Shard + fold weights. Returns (in_maps, meta)."""
    f64 = np.float64

    heads = np.asarray(edge_index[0], dtype=np.int64)
    tails = np.asarray(edge_index[1], dtype=np.int64)
    attrs = np.asarray(edge_attr, dtype=np.int64)

    # ---- per-core edge bucketing by tail group --------------------------
    core_of = tails // NODES_PER_CORE
    core_of = np.minimum(core_of, N_CORES - 1)
    per_core = []
    K_max = 1
    for c in range(N_CORES):
        m = core_of == c
        h_c, t_c, a_c = heads[m], tails[m], attrs[m]
        g_c = (t_c - c * NODES_PER_CORE) // GROUP
        loc_c = (t_c - c * NODES_PER_CORE) % GROUP
        order = np.argsort(g_c, kind="stable")
        h_c, a_c, g_c, loc_c = h_c[order], a_c[order], g_c[order], loc_c[order]
        counts = np.bincount(g_c, minlength=NG)
        K_max = max(K_max, int(np.max((counts + 127) // 128)))
        per_core.append((h_c, a_c, g_c, loc_c, counts))

    K = K_max
    NT = NG * K            # tiles per core
    heads_np = np.zeros((N_CORES, NT, 128), np.int32)
    attrs_np = np.zeros((N_CORES, NT, 128), np.int32)
    tails_np = np.full((N_CORES, NG, 128, K), 255.0, np.float32)
    for c in range(N_CORES):
        h_c, a_c, g_c, loc_c, counts = per_core[c]
        off = 0
        for g in range(NG):
            n = int(counts[g])
            hg, ag, lg = h_c[off:off+n], a_c[off:off+n], loc_c[off:off+n]
            off += n
            base_t = g * K
            hh = np.zeros(K * 128, np.int32)
            aa = np.zeros(K * 128, np.int32)
            tt = np.full(K * 128, 255.0, np.float32)
            hh[:n] = hg
            aa[:n] = ag
            tt[:n] = lg.astype(np.float32)
            heads_np[c, base_t:base_t+K] = hh.reshape(K, 128)
            attrs_np[c, base_t:base_t+K] = aa.reshape(K, 128)
            tails_np[c, g] = tt.reshape(K, 128).T

    # ---- weight folding (fp64) -----------------------------------------
    W_mess = np.asarray(W_mess, f64); b_mess = np.asarray(b_mess, f64)
    W_matt = np.asarray(W_matt, f64); b_matt = np.asarray(b_matt, f64)
    W_xatt = np.asarray(W_xatt, f64); b_xatt = np.asarray(b_xatt, f64)
    a = np.asarray(mess_atten_weight, f64)[0]
    ax = np.asarray(x_atten_weight, f64)[0]
    qc = np.asarray(que_context, f64)[0]
    fq = np.asarray(fin_que, f64)[0, 0]

    Wm1 = W_mess[:, :HID]; Wm2 = W_mess[:, HID:]
    W1 = W_matt[:, :HID]
    bias_m = W_matt[:, HID:] @ qc + b_matt          # [128]
    Wx1 = W_xatt[:, :HID]
    bias_x = W_xatt[:, HID:] @ fq + b_xatt          # [128]

    def fold(Wf, biasf, af):
        s = np.where(af >= 0, 1.0, -1.0)
        absa = np.abs(af)
        Was = absa[:, None] * Wf                    # [h', h]
        bias_as = absa * biasf                      # [h']
        u = Wf.T @ af                               # [h]
        cz = float(af @ biasf)
        return Was, bias_as, s, u, cz

    W1as, bias_as, s_m, u2, cz = fold(W1, bias_m, a)
    Wx1as, bias_xas, s_x, ux, cxz = fold(Wx1, bias_x, ax)

    def bc(row, dt=np.float32):
        return np.tile(np.asarray(row, np.float64)[None, :], (128, 1)).astype(dt)

    weights = {
        "Wm1T": Wm1.T.astype(bf16),                     # [h, h'] rhs for xW
        "Wm2T": Wm2.T.astype(bf16),
        "bmess_bc": bc(b_mess),
        "W1asT": W1as.T.astype(bf16),                   # [h, h'] lhsT att
        "bias_as": bias_as.astype(np.float32)[:, None],  # [h',1]
        "spm_bc": bc(s_m, bf16),
        "u2_bc": bc(u2 * (LRELU / (1 - LRELU)), bf16),
        "Wx1asT": Wx1as.T.astype(bf16),
        "bias_xas": bias_xas.astype(np.float32)[:, None],
        "spmx_bc": bc(s_x, bf16),
        "ux_bc": bc(ux * (LRELU / (1 - LRELU)), bf16),
        "iota_bc": np.tile(np.arange(128, dtype=np.float64)[None, :],
                           (128, 1)).astype(bf16),
        "ones_col": np.ones((128, 1), bf16),
        "ones_row_f": np.ones((1, 128), np.float32),
        "gamma_row": np.asarray(bn_gamma, np.float64)[None, :].astype(np.float32),
        "beta_row": np.asarray(bn_beta, np.float64)[None, :].astype(np.float32),
    }
    consts = {"cz": cz * (1 - LRELU) * LRELU / (1 - LRELU),  # placeholder, fixed below
              }
    # c = 0.99*(t1+t2) + 0.01*cz  -> exp(c): ACT exp(scale*in + bias)
    consts["exp_scale"] = (1 - LRELU)
    consts["exp_bias"] = LRELU * cz
    consts["sig_scale"] = (1 - LRELU)
    consts["sig_bias"] = 0.0   # cxz cancels in l0 - l1

    x_np = np.asarray(x, np.float32)
    x_pad = np.zeros((X_PAD, HID), np.float32)
    x_pad[:N_NODES] = x_np

    in_maps = []
    for c in range(N_CORES):
        xs = np.zeros((NODES_PAD, HID), np.float32)
        lo = c * NODES_PER_CORE
        hi = min(lo + NODES_PER_CORE, N_NODES)
        xs[:hi - lo] = x_np[lo:hi]
        m = {
            "x_full": x_pad,
            "x_slice": xs,
            "r_in": np.asarray(r, np.float32),
            "heads": heads_np[c][:, :, None],
            "attrs": attrs_np[c][:, :, None],
            "tails": tails_np[c],
            "ones_colf": np.ones((128, 1), np.float32),
        }
        m.update({k: np.ascontiguousarray(v) for k, v in weights.items()})
        in_maps.append(m)

    meta = {"K": K, "NT": NT, "consts": consts}
    return in_maps, meta


def _build(meta):
    import concourse.bass as bass
    import concourse.bacc as bacc
    import concourse.mybir as mybir
    import concourse.tile as tile
    from concourse.masks import make_identity

    dt = mybir.dt
    K = meta["K"]; NT = meta["NT"]; C = meta["consts"]
    nc = bacc.Bacc()

    # ---------------- I/O ------------------------------------------------
    x_full = nc.dram_tensor("x_full", [X_PAD, HID], dt.float32, kind="ExternalInput")
    x_slice = nc.dram_tensor("x_slice", [NODES_PAD, HID], dt.float32, kind="ExternalInput")
    r_in = nc.dram_tensor("r_in", [N_REL, HID], dt.float32, kind="ExternalInput")
    heads = nc.dram_tensor("heads", [NT, 128, 1], dt.int32, kind="ExternalInput")
    attrs = nc.dram_tensor("attrs", [NT, 128, 1], dt.int32, kind="ExternalInput")
    tails = nc.dram_tensor("tails", [NG, 128, K], dt.float32, kind="ExternalInput")
    ones_colf = nc.dram_tensor("ones_colf", [128, 1], dt.float32, kind="ExternalInput")

    def win(name, shape, d=dt.float32):
        return nc.dram_tensor(name, shape, d, kind="ExternalInput")

    Wm1T = win("Wm1T", [HID, HID], dt.bfloat16)
    Wm2T = win("Wm2T", [HID, HID], dt.bfloat16)
    bmess_bc = win("bmess_bc", [128, 128])
    W1asT = win("W1asT", [HID, HID], dt.bfloat16)
    bias_as = win("bias_as", [128, 1])
    spm_bc = win("spm_bc", [128, 128], dt.bfloat16)
    u2_bc = win("u2_bc", [128, 128], dt.bfloat16)
    Wx1asT = win("Wx1asT", [HID, HID], dt.bfloat16)
    bias_xas = win("bias_xas", [128, 1])
    spmx_bc = win("spmx_bc", [128, 128], dt.bfloat16)
    ux_bc = win("ux_bc", [128, 128], dt.bfloat16)
    iota_bc = win("iota_bc", [128, 128], dt.bfloat16)
    ones_col = win("ones_col", [128, 1], dt.bfloat16)
    ones_row_f = win("ones_row_f", [1, 128])
    gamma_row = win("gamma_row", [1, 128])
    beta_row = win("beta_row", [1, 128])

    x_out = nc.dram_tensor("x_out", [NODES_PAD, HID], dt.float32, kind="ExternalOutput")
    r_out = nc.dram_tensor("r_out", [N_REL, HID], dt.float32, kind="ExternalOutput")

    xW = nc.dram_tensor("xW_tab", [X_PAD, HID], dt.bfloat16, kind="Internal")
    rWb = nc.dram_tensor("rWb_tab", [512, HID], dt.bfloat16, kind="Internal")

    with tile.TileContext(nc) as tc:
        with tc.tile_pool(name="cst", bufs=1) as cst, \
             tc.tile_pool(name="sb", bufs=3) as sb, \
             tc.tile_pool(name="ps", bufs=1, space="PSUM") as ps, \
             tc.tile_pool(name="psU", bufs=2, space="PSUM") as psU:

            # ---- constants resident in SBUF -----------------------------
            def ld_const(t, d=dt.float32, shape=(128, 128)):
                tl = cst.tile(list(shape), d, tag=t.name)
                nc.sync.dma_start(tl[:], t[:])
                return tl

            Wm1T_t = ld_const(Wm1T, dt.bfloat16)
            Wm2T_t = ld_const(Wm2T, dt.bfloat16)
            bmess_t = ld_const(bmess_bc)
            W1asT_t = ld_const(W1asT, dt.bfloat16)
            bias_as_t = ld_const(bias_as, dt.float32, (128, 1))
            spm_t = ld_const(spm_bc, dt.bfloat16)
            u2_t = ld_const(u2_bc, dt.bfloat16)
            Wx1asT_t = ld_const(Wx1asT, dt.bfloat16)
            bias_xas_t = ld_const(bias_xas, dt.float32, (128, 1))
            spmx_t = ld_const(spmx_bc, dt.bfloat16)
            ux_t = ld_const(ux_bc, dt.bfloat16)
            iota_t = ld_const(iota_bc, dt.bfloat16)
            ones_t = ld_const(ones_col, dt.bfloat16, (128, 1))
            onesrow_t = ld_const(ones_row_f, dt.float32, (1, 128))
            onescf_t = ld_const(ones_colf, dt.float32, (128, 1))
            gamma_t = ld_const(gamma_row, dt.float32, (1, 128))
            beta_t = ld_const(beta_row, dt.float32, (1, 128))
            ident_t = cst.tile([128, 128], dt.float32, tag="ident")
            make_identity(nc, ident_t[:])

            # ================= phase 1: BN(r) + rWb table ================
            # column sums of r and r^2 via ones-matmul accumulation
            stat_ps = ps.tile([1, 256], dt.float32, tag="acc", bufs=1)
            rsq = sb.tile([128, 128], dt.float32, tag="rsq")
            rchunks = []
            for i in range(4):
                rows = min(128, N_REL - i * 128)
                rt = sb.tile([128, 128], dt.float32, tag=f"rch{i}")
                nc.sync.dma_start(rt[:rows, :], r_in[i*128:i*128+rows, :])
                rchunks.append((rt, rows))
                nc.tensor.matmul(stat_ps[:1, 0:128], lhsT=onescf_t[:rows, :],
                                 rhs=rt[:rows, :], start=(i == 0), stop=(i == 3))
            # squares accumulated separately
            for i in range(4):
                rt, rows = rchunks[i]
                nc.scalar.square(rsq[:rows, :], rt[:rows, :])
                nc.tensor.matmul(stat_ps[:1, 128:256], lhsT=onescf_t[:rows, :],
                                 rhs=rsq[:rows, :], start=(i == 0), stop=(i == 3))
            mu = sb.tile([1, 128], dt.float32, tag="mu")
            nc.scalar.mul(mu[:], stat_ps[:1, 0:128], 1.0 / N_REL)
            var = sb.tile([1, 128], dt.float32, tag="var")
            nc.scalar.mul(var[:], stat_ps[:1, 128:256], 1.0 / N_REL)
            musq = sb.tile([1, 128], dt.float32, tag="musq")
            nc.vector.tensor_mul(musq[:], mu[:], mu[:])
            nc.vector.tensor_sub(var[:], var[:], musq[:])
            # scale = gamma * rsqrt(var + eps)
            nc.vector.tensor_scalar_add(var[:], var[:], BN_EPS)
            nc.scalar.activation(var[:], var[:], mybir.ActivationFunctionType.Sqrt)
            rinv = sb.tile([1, 128], dt.float32, tag="rinv")
            nc.vector.reciprocal(rinv[:], var[:])
            scale_r = sb.tile([1, 128], dt.float32, tag="scale_r")
            nc.vector.tensor_mul(scale_r[:], gamma_t[:], rinv[:])
            shift_r = sb.tile([1, 128], dt.float32, tag="shift_r")
            nc.vector.tensor_mul(shift_r[:], mu[:], scale_r[:])
            nc.vector.tensor_sub(shift_r[:], beta_t[:], shift_r[:])
            # broadcast rows to [128,128] via K=1 matmul
            bcast_ps = ps.tile([128, 256], dt.float32, tag="acc", bufs=1)
            nc.tensor.matmul(bcast_ps[:, 0:128], lhsT=onesrow_t[:], rhs=scale_r[:],
                             start=True, stop=True)
            nc.tensor.matmul(bcast_ps[:, 128:256], lhsT=onesrow_t[:], rhs=shift_r[:],
                             start=True, stop=True)
            scale_bc = sb.tile([128, 128], dt.float32, tag="scale_bc")
            nc.vector.tensor_copy(scale_bc[:], bcast_ps[:, 0:128])
            shift_bc = sb.tile([128, 128], dt.float32, tag="shift_bc")
            nc.vector.tensor_copy(shift_bc[:], bcast_ps[:, 128:256])

            for i in range(4):
                rt, rows = rchunks[i]
                rn = sb.tile([128, 128], dt.float32, tag="rn")
                nc.vector.tensor_mul(rn[:rows, :], rt[:rows, :], scale_bc[:rows, :])
                nc.vector.tensor_add(rn[:rows, :], rn[:rows, :], shift_bc[:rows, :])
                nc.sync.dma_start(r_out[i*128:i*128+rows, :], rn[:rows, :])
                # rWb chunk: transpose rn, matmul with Wm2T, add b_mess
                tp = ps.tile([128, 256], dt.float32, tag="trans", bufs=2)
                nc.tensor.transpose(tp[:, :rows], rn[:rows, :], ident_t[:rows, :rows])
                rnT = sb.tile([128, 128], dt.bfloat16, tag="rnT")
                nc.vector.tensor_copy(rnT[:, :rows], tp[:, :rows])
                mm = ps.tile([128, 256], dt.float32, tag="trans", bufs=2)
                nc.tensor.matmul(mm[:rows, :128], lhsT=rnT[:, :rows], rhs=Wm2T_t[:],
                                 start=True, stop=True)
                rwb = sb.tile([128, 128], dt.bfloat16, tag="rwb")
                nc.vector.tensor_add(rwb[:rows, :], mm[:rows, :128], bmess_t[:rows, :])
                nc.sync.dma_start(rWb[i*128:i*128+rows, :], rwb[:rows, :])
            zpad = sb.tile([12, 128], dt.bfloat16, tag="zpad")
            nc.vector.memset(zpad[:], 0.0)
            nc.sync.dma_start(rWb[500:512, :], zpad[:])

            # ================= phase 0: xW table ========================
            for i in range(X_CHUNKS if CUT != 1 else 0):
                xb = sb.tile([128, 128], dt.bfloat16, tag="p0xb")
                nc.gpsimd.dma_start(xb[:], x_full[i*128:(i+1)*128, :])  # cast f32->bf16
                xbT = sb.tile([128, 128], dt.bfloat16, tag="p0xbT")
                nc.sync.dma_start(xbT[:], xb[:], transpose=True)
                mm = ps.tile([128, 256], dt.float32, tag="trans", bufs=2)
                nc.tensor.matmul(mm[:, :128], lhsT=xbT[:], rhs=Wm1T_t[:], start=True, stop=True)
                ob = sb.tile([128, 128], dt.bfloat16, tag="p0ob")
                nc.any.tensor_copy(ob[:], mm[:, :128])
                nc.sync.dma_start(xW[i*128:(i+1)*128, :], ob[:])

            # ================= phase 2: edges ===========================
            SUB = 7 if K >= 7 else K
            n_sub = (K + SUB - 1) // SUB
            for rep_i in range(REPEAT):
              for g in range(NG if CUT not in (1, 2) else 0):
                U = psU.tile([128, 132], dt.float32, tag="U")
                w_g = sb.tile([128, K], dt.float32, tag="w_g")
                tails_t = sb.tile([128, K], dt.float32, tag="tails_t")
                nc.sync.dma_start(tails_t[:], tails[g])
                mess_g = sb.tile([128, K, 132], dt.bfloat16, tag="mess_g")
                nc.vector.memset(mess_g[:, :, 128:129], 1.0)
                for si in range(n_sub):
                    k0 = si * SUB
                    k1 = min(K, k0 + SUB)
                    nk = k1 - k0
                    pre = sb.tile([128, SUB * 128], dt.bfloat16, tag="pre")
                    for kk in range(k0, k1):
                        t_idx = g * K + kk
                        j = kk - k0
                        it = sb.tile([128, 1], dt.int32, tag="it")
                        nc.sync.dma_start(it[:], heads[t_idx])
                        nc.gpsimd.indirect_dma_start(
                            out=pre[:, j*128:(j+1)*128], out_offset=None,
                            in_=xW[:],
                            in_offset=bass.IndirectOffsetOnAxis(ap=it[:], axis=0))
                        at = sb.tile([128, 1], dt.int32, tag="at")
                        nc.sync.dma_start(at[:], attrs[t_idx])
                        if USE_CCE:
                            nc.gpsimd.indirect_dma_start(
                                out=pre[:, j*128:(j+1)*128], out_offset=None,
                                in_=rWb[:],
                                in_offset=bass.IndirectOffsetOnAxis(ap=at[:], axis=0),
                                compute_op=mybir.AluOpType.add)
                        else:
                            rb = sb.tile([128, 128], dt.bfloat16, tag="rb")
                            nc.gpsimd.indirect_dma_start(
                                out=rb[:], out_offset=None,
                                in_=rWb[:],
                                in_offset=bass.IndirectOffsetOnAxis(ap=at[:], axis=0))
                            nc.vector.tensor_add(pre[:, j*128:(j+1)*128],
                                                 pre[:, j*128:(j+1)*128], rb[:])
                    if STAGE < 1:
                        continue
                    # tanh
                    mview = mess_g[:, k0:k0+nk, :128]
                    nc.scalar.activation(mview, pre[:, :nk*128].rearrange(
                        "p (k f) -> p k f", f=128),
                                         mybir.ActivationFunctionType.Tanh)
                    if STAGE < 2:
                        continue
                    # transpose each tile
                    msT = sb.tile([128, SUB * 128], dt.bfloat16, tag="msT")
                    for j in range(nk):
                        nc.sync.dma_start(msT[:, j*128:(j+1)*128],
                                          mess_g[:, k0+j, :128],
                                          transpose=True)
                    # attention matmul [h' x nk*128]
                    vps = ps.tile([128, SUB * 128], dt.float32, tag="vps", bufs=1)
                    for mm0 in range(0, nk*128, 512):
                        mm1 = min(nk*128, mm0 + 512)
                        nc.tensor.matmul(vps[:, mm0:mm1], lhsT=W1asT_t[:],
                                         rhs=msT[:, mm0:mm1], start=True, stop=True)
                    relu = sb.tile([128, SUB * 128], dt.bfloat16, tag="relu")
                    nc.vector.tensor_scalar(
                        out=relu[:, :nk*128], in0=vps[:, :nk*128],
                        scalar1=bias_as_t[:], scalar2=0.0,
                        op0=mybir.AluOpType.add, op1=mybir.AluOpType.max)
                    if STAGE < 3:
                        continue
                    # transpose relu tiles back; reduce
                    ruT = sb.tile([128, SUB * 128], dt.bfloat16, tag="ruT")
                    for j in range(nk):
                        nc.sync.dma_start(ruT[:, j*128:(j+1)*128],
                                          relu[:, j*128:(j+1)*128], transpose=True)
                    t2s = sb.tile([128, SUB], dt.float32, tag="t2s")
                    t1s = sb.tile([128, SUB], dt.float32, tag="t1s")
                    scratch = sb.tile([128, 256], dt.float32, tag="ttr_scr")
                    for j in range(nk):
                        nc.vector.tensor_mul(scratch[:, 0:128],
                                             ruT[:, j*128:(j+1)*128], spm_t[:])
                        nc.vector.reduce_sum(t2s[:, j:j+1], scratch[:, 0:128],
                                             axis=mybir.AxisListType.X)
                        nc.vector.tensor_mul(scratch[:, 128:256],
                                             mess_g[:, k0+j, :128], u2_t[:])
                        nc.vector.reduce_sum(t1s[:, j:j+1], scratch[:, 128:256],
                                             axis=mybir.AxisListType.X)
                    csum = sb.tile([128, SUB], dt.float32, tag="csum")
                    nc.vector.tensor_add(csum[:, :nk], t2s[:, :nk], t1s[:, :nk])
                    # exp bias (0.01*cz) is uniform across edges -> cancels in
                    # the segment softmax normalization; drop it.
                    nc.scalar.activation(w_g[:, k0:k1], csum[:, :nk],
                                         mybir.ActivationFunctionType.Exp,
                                         scale=float(C["exp_scale"]))
                    if STAGE < 4:
                        continue
                    # scatter matmuls
                    for j in range(nk):
                        kk = k0 + j
                        sw = sb.tile([128, 128], dt.bfloat16, tag="sw")
                        nc.vector.tensor_scalar(
                            out=sw[:], in0=iota_t[:],
                            scalar1=tails_t[:, kk:kk+1], scalar2=w_g[:, kk:kk+1],
                            op0=mybir.AluOpType.is_equal, op1=mybir.AluOpType.mult)
                        nc.tensor.matmul(U[:, 0:129], lhsT=sw[:],
                                         rhs=mess_g[:, kk, :129],
                                         start=(kk == 0), stop=(kk == K - 1))

                if STAGE < 5:
                    continue
                # ---- normalize + gating for this group -----------------
                zc = sb.tile([128, 1], dt.float32, tag="zc")
                nc.vector.tensor_scalar_add(zc[:], U[:, 128:129], 1e-16)
                rec = sb.tile([128, 1], dt.float32, tag="rec")
                nc.vector.reciprocal(rec[:], zc[:])
                sm_f = sb.tile([128, 128], dt.float32, tag="sm_f")
                nc.vector.tensor_scalar_mul(sm_f[:], U[:, 0:128], rec[:])
                sm_b = sb.tile([128, 128], dt.bfloat16, tag="sm_b")
                nc.vector.tensor_copy(sm_b[:], sm_f[:])
                xg = sb.tile([128, 128], dt.float32, tag="xg")
                nc.sync.dma_start(xg[:], x_slice[g*128:(g+1)*128, :])
                xg_b = sb.tile([128, 128], dt.bfloat16, tag="xg_b")
                nc.vector.tensor_copy(xg_b[:], xg[:])
                vT = sb.tile([128, 256], dt.bfloat16, tag="vT")
                nc.sync.dma_start(vT[:, 0:128], xg_b[:], transpose=True)
                nc.sync.dma_start(vT[:, 128:256], sm_b[:], transpose=True)
                gps = ps.tile([128, 256], dt.float32, tag="trans", bufs=2)
                nc.tensor.matmul(gps[:], lhsT=Wx1asT_t[:], rhs=vT[:],
                                 start=True, stop=True)
                grelu = sb.tile([128, 256], dt.bfloat16, tag="grelu")
                nc.vector.tensor_scalar(
                    out=grelu[:], in0=gps[:],
                    scalar1=bias_xas_t[:], scalar2=0.0,
                    op0=mybir.AluOpType.add, op1=mybir.AluOpType.max)
                gruT = sb.tile([128, 256], dt.bfloat16, tag="gruT")
                nc.sync.dma_start(gruT[:, 0:128], grelu[:, 0:128], transpose=True)
                nc.sync.dma_start(gruT[:, 128:256], grelu[:, 128:256], transpose=True)
                gt = sb.tile([128, 4], dt.float32, tag="gt")
                gscr = sb.tile([128, 128], dt.float32, tag="gscr")
                nc.vector.tensor_mul(gscr[:], gruT[:, 0:128], spmx_t[:])
                nc.vector.reduce_sum(gt[:, 0:1], gscr[:],
                                     axis=mybir.AxisListType.X)
                nc.vector.tensor_mul(gscr[:], gruT[:, 128:256], spmx_t[:])
                nc.vector.reduce_sum(gt[:, 1:2], gscr[:],
                                     axis=mybir.AxisListType.X)
                # t1 terms: (x - sm) . ux
                dxm = sb.tile([128, 128], dt.float32, tag="dxm")
                nc.vector.tensor_sub(dxm[:], xg[:], sm_f[:])
                nc.vector.tensor_mul(gscr[:], dxm[:], ux_t[:])
                nc.vector.reduce_sum(gt[:, 2:3], gscr[:],
                                     axis=mybir.AxisListType.X)
                # delta = t2x0 - t2x1 + t1x
                nc.vector.tensor_sub(gt[:, 3:4], gt[:, 0:1], gt[:, 1:2])
                nc.vector.tensor_add(gt[:, 3:4], gt[:, 3:4], gt[:, 2:3])
                w0 = sb.tile([128, 1], dt.float32, tag="w0")
                nc.scalar.activation(w0[:], gt[:, 3:4],
                                     mybir.ActivationFunctionType.Sigmoid,
                                     scale=float(C["sig_scale"]))
                # x_out = sm + w0 * (x - sm)
                xo = sb.tile([128, 128], dt.float32, tag="xo")
                nc.vector.tensor_scalar_mul(xo[:], dxm[:], w0[:])
                nc.vector.tensor_add(xo[:], xo[:], sm_f[:])
                nc.sync.dma_start(x_out[g*128:(g+1)*128, :], xo[:])

        if CUT in (1, 2):
            with tc.tile_pool(name="zfill", bufs=1) as zf:
                zt = zf.tile([128, 128], dt.float32)
                nc.vector.memset(zt[:], 0.0)
                for g in range(NG):
                    nc.sync.dma_start(x_out[g*128:(g+1)*128, :], zt[:])

    nc.finalize()
    return nc


def kernel(**inputs):
    from concourse.bass_utils import run_bass_kernel_spmd

    in_maps, meta = _host_prep(**inputs)
    key = meta["K"]
    if key not in _COMPILED:
        _COMPILED[key] = _build(meta)
    nc = _COMPILED[key]
    res = run_bass_kernel_spmd(nc, in_maps, core_ids=list(range(N_CORES)))
    x_parts = [res.results[c]["x_out"][:NODES_PER_CORE] for c in range(N_CORES)]
    x_out = np.concatenate(x_parts, axis=0)
    r_out = res.results[0]["r_out"]
    return x_out, r_out


if __name__ == "__main__":
    import reference
    inputs = {k: np.asarray(v) for k, v in reference.setup_inputs().items()}
    got_x, got_r = kernel(**inputs)
    exp_x, exp_r = reference.reference(**reference.setup_inputs())
    exp_x = np.asarray(exp_x); exp_r = np.asarray(exp_r)
    for name, g, e in (("x_out", got_x, exp_x), ("r", got_r, exp_r)):
        rel = np.abs(g - e).max() / (np.abs(e).max() + 1e-9)
        print(f"{name}: max abs {np.abs(g-e).max():.5g}  rel {rel:.5g}")


# revision 20
# speedup vs baseline: 1.5607x; 1.5607x over previous
"""AttenGcnLayer Trainium2 kernel (8 NeuronCores, SPMD).

Strategy: edges are bucketed on host by destination (tail) node; nodes are
range-partitioned across the 8 cores, so every edge is device-local to the
core owning its tail (no collectives). Per-node-group segment softmax and
weighted scatter-sum are computed with one-hot matmuls accumulated in PSUM.

Algebraic restructuring (all weight-folding done on host in fp64):
  mess      = tanh(xW[head] + rWb[attr]) with xW = x @ Wm1.T (device phase-0,
              bf16 table in HBM), rWb = BN(r) @ Wm2.T + b_mess (device, tiny)
  att logit c = sum_h' a*lrelu(mess @ W1.T + bias)
            = 0.99*(t1 + t2) + 0.01*cz
    t1 = (0.01/0.99) * mess . u2            (u2 = W1.T a; DVE row-reduce)
    t2 = sum_h' s_h' * relu(mess @ W1as.T + bias_as)  (sign-folded weights;
         relu is a single fused DVE tensor_scalar op; sign sum via +-1 mask)
  softmax over segments: exp without max-subtraction (logits ~ N(0,1)),
  normalization U/(Z+eps) from an extra ones-column in the scatter matmul.
  gating softmax over 2 = sigmoid of logit difference, same relu-fold.
"""

import sys
sys.path.insert(0, "/opt/trn_rl_repo")

import numpy as np
import ml_dtypes

HID = 128
N_NODES = 50000
N_REL = 500
N_EDGES = 625000
N_CORES = 8
BN_EPS = 1e-5
NODES_PER_CORE = N_NODES // N_CORES      # 6250
GROUP = 128
NG = (NODES_PER_CORE + GROUP - 1) // GROUP   # 49 groups/core
NODES_PAD = NG * GROUP                        # 6272
X_CHUNKS = (N_NODES + 127) // 128             # 391
X_PAD = X_CHUNKS * 128                        # 50048
LRELU = 0.01

bf16 = ml_dtypes.bfloat16
USE_CCE = True
import os
CUT = int(os.environ.get("KERNEL_CUT", "0"))
STAGE = int(os.environ.get("KERNEL_STAGE", "9"))
REPEAT = int(os.environ.get("KERNEL_REPEAT", "1"))  # 0=full, 1=phase1 only, 2=+phase0, 3=+edges-no-gating

_COMPILED = {}


def _host_prep(x, r, que_context, fin_que, edge_index, edge_attr,
               W_mess, b_mess, W_matt, b_matt, mess_atten_weight,
               W_xatt, b_xatt, x_atten_weight, bn_gamma, bn_beta):
    """Shard + fold weights. Returns (in_maps, meta)."""
    f64 = np.float64

    heads = np.asarray(edge_index[0], dtype=np.int64)
    tails = np.asarray(edge_index[1], dtype=np.int64)
    attrs = np.asarray(edge_attr, dtype=np.int64)

    # ---- per-core edge bucketing by tail group --------------------------
    core_of = tails // NODES_PER_CORE
    core_of = np.minimum(core_of, N_CORES - 1)
    per_core = []
    K_max = 1
    for c in range(N_CORES):
        m = core_of == c
        h_c, t_c, a_c = heads[m], tails[m], attrs[m]
        g_c = (t_c - c * NODES_PER_CORE) // GROUP
        loc_c = (t_c - c * NODES_PER_CORE) % GROUP
        order = np.argsort(g_c, kind="stable")
        h_c, a_c, g_c, loc_c = h_c[order], a_c[order], g_c[order], loc_c[order]
        counts = np.bincount(g_c, minlength=NG)
        K_max = max(K_max, int(np.max((counts + 127) // 128)))
        per_core.append((h_c, a_c, g_c, loc_c, counts))

    K = K_max
    NT = NG * K            # tiles per core
    heads_np = np.zeros((N_CORES, NT, 128), np.int32)
    attrs_np = np.zeros((N_CORES, NT, 128), np.int32)
    tails_np = np.full((N_CORES, NG, 128, K), 255.0, np.float32)
    for c in range(N_CORES):
        h_c, a_c, g_c, loc_c, counts = per_core[c]
        off = 0
        for g in range(NG):
            n = int(counts[g])
            hg, ag, lg = h_c[off:off+n], a_c[off:off+n], loc_c[off:off+n]
            off += n
            base_t = g * K
            hh = np.zeros(K * 128, np.int32)
            aa = np.zeros(K * 128, np.int32)
            tt = np.full(K * 128, 255.0, np.float32)
            hh[:n] = hg
            aa[:n] = ag
            tt[:n] = lg.astype(np.float32)
            heads_np[c, base_t:base_t+K] = hh.reshape(K, 128)
            attrs_np[c, base_t:base_t+K] = aa.reshape(K, 128)
            tails_np[c, g] = tt.reshape(K, 128).T

    # ---- weight folding (fp64) -----------------------------------------
    W_mess = np.asarray(W_mess, f64); b_mess = np.asarray(b_mess, f64)
    W_matt = np.asarray(W_matt, f64); b_matt = np.asarray(b_matt, f64)
    W_xatt = np.asarray(W_xatt, f64); b_xatt = np.asarray(b_xatt, f64)
    a = np.asarray(mess_atten_weight, f64)[0]
    ax = np.asarray(x_atten_weight, f64)[0]
    qc = np.asarray(que_context, f64)[0]
    fq = np.asarray(fin_que, f64)[0, 0]

    Wm1 = W_mess[:, :HID]; Wm2 = W_mess[:, HID:]
    W1 = W_matt[:, :HID]
    bias_m = W_matt[:, HID:] @ qc + b_matt          # [128]
    Wx1 = W_xatt[:, :HID]
    bias_x = W_xatt[:, HID:] @ fq + b_xatt          # [128]

    def fold(Wf, biasf, af):
        s = np.where(af >= 0, 1.0, -1.0)
        absa = np.abs(af)
        Was = absa[:, None] * Wf                    # [h', h]
        bias_as = absa * biasf                      # [h']
        u = Wf.T @ af                               # [h]
        cz = float(af @ biasf)
        return Was, bias_as, s, u, cz

    W1as, bias_as, s_m, u2, cz = fold(W1, bias_m, a)
    Wx1as, bias_xas, s_x, ux, cxz = fold(Wx1, bias_x, ax)

    def bc(row, dt=np.float32):
        return np.tile(np.asarray(row, np.float64)[None, :], (128, 1)).astype(dt)

    weights = {
        "Wm1T": Wm1.T.astype(bf16),                     # [h, h'] rhs for xW
        "Wm2T": Wm2.T.astype(bf16),
        "bmess_bc": bc(b_mess),
        "W1asT": W1as.T.astype(bf16),                   # [h, h'] lhsT att
        "bias_as": bias_as.astype(np.float32)[:, None],  # [h',1]
        "spm_col": s_m.astype(bf16)[:, None],
        "u2s_col": (u2 * (LRELU / (1 - LRELU))).astype(bf16)[:, None],
        "Wx1asT": Wx1as.T.astype(bf16),
        "bias_xas": bias_xas.astype(np.float32)[:, None],
        "spmx_col": s_x.astype(bf16)[:, None],
        "ux_col": (ux * (LRELU / (1 - LRELU))).astype(bf16)[:, None],
        "iota_bc": np.tile(np.arange(128, dtype=np.float64)[None, :],
                           (128, 1)).astype(bf16),
        "ones_col": np.ones((128, 1), bf16),
        "ones_row_f": np.ones((1, 128), np.float32),
        "gamma_row": np.asarray(bn_gamma, np.float64)[None, :].astype(np.float32),
        "beta_row": np.asarray(bn_beta, np.float64)[None, :].astype(np.float32),
    }
    consts = {"cz": cz * (1 - LRELU) * LRELU / (1 - LRELU),  # placeholder, fixed below
              }
    # c = 0.99*(t1+t2) + 0.01*cz  -> exp(c): ACT exp(scale*in + bias)
    consts["exp_scale"] = (1 - LRELU)
    consts["exp_bias"] = LRELU * cz
    consts["sig_scale"] = (1 - LRELU)
    consts["sig_bias"] = 0.0   # cxz cancels in l0 - l1

    x_np = np.asarray(x, np.float32)
    xT_pad = np.zeros((HID, X_PAD), bf16)
    xT_pad[:, :N_NODES] = x_np.T.astype(bf16)

    in_maps = []
    for c in range(N_CORES):
        xs = np.zeros((NODES_PAD, HID), np.float32)
        lo = c * NODES_PER_CORE
        hi = min(lo + NODES_PER_CORE, N_NODES)
        xs[:hi - lo] = x_np[lo:hi]
        m = {
            "xT_full": xT_pad,
            "x_slice": xs,
            "xT_slice": np.ascontiguousarray(xs.T.astype(bf16)),
            "r_in": np.asarray(r, np.float32),
            "headsT": np.ascontiguousarray(heads_np[c].T),
            "attrsT": np.ascontiguousarray(attrs_np[c].T),
            "tailsT": np.ascontiguousarray(
                tails_np[c].transpose(1, 0, 2).reshape(128, NG * K)),
            "ones_colf": np.ones((128, 1), np.float32),
        }
        m.update({k: np.ascontiguousarray(v) for k, v in weights.items()})
        in_maps.append(m)

    meta = {"K": K, "NT": NT, "consts": consts}
    return in_maps, meta


def _build(meta):
    import concourse.bass as bass
    import concourse.bacc as bacc
    import concourse.mybir as mybir
    import concourse.tile as tile
    from concourse.masks import make_identity

    dt = mybir.dt
    K = meta["K"]; NT = meta["NT"]; C = meta["consts"]
    nc = bacc.Bacc()

    # ---------------- I/O ------------------------------------------------
    xT_full = nc.dram_tensor("xT_full", [HID, X_PAD], dt.bfloat16, kind="ExternalInput")
    x_slice = nc.dram_tensor("x_slice", [NODES_PAD, HID], dt.float32, kind="ExternalInput")
    xT_slice = nc.dram_tensor("xT_slice", [HID, NODES_PAD], dt.bfloat16, kind="ExternalInput")
    r_in = nc.dram_tensor("r_in", [N_REL, HID], dt.float32, kind="ExternalInput")
    headsT = nc.dram_tensor("headsT", [128, NT], dt.int32, kind="ExternalInput")
    attrsT = nc.dram_tensor("attrsT", [128, NT], dt.int32, kind="ExternalInput")
    tailsT = nc.dram_tensor("tailsT", [128, NT], dt.float32, kind="ExternalInput")
    ones_colf = nc.dram_tensor("ones_colf", [128, 1], dt.float32, kind="ExternalInput")

    def win(name, shape, d=dt.float32):
        return nc.dram_tensor(name, shape, d, kind="ExternalInput")

    Wm1T = win("Wm1T", [HID, HID], dt.bfloat16)
    Wm2T = win("Wm2T", [HID, HID], dt.bfloat16)
    bmess_bc = win("bmess_bc", [128, 128])
    W1asT = win("W1asT", [HID, HID], dt.bfloat16)
    bias_as = win("bias_as", [128, 1])
    spm_col = win("spm_col", [128, 1], dt.bfloat16)
    u2s_col = win("u2s_col", [128, 1], dt.bfloat16)
    Wx1asT = win("Wx1asT", [HID, HID], dt.bfloat16)
    bias_xas = win("bias_xas", [128, 1])
    spmx_col = win("spmx_col", [128, 1], dt.bfloat16)
    ux_col = win("ux_col", [128, 1], dt.bfloat16)
    iota_bc = win("iota_bc", [128, 128], dt.bfloat16)
    ones_col = win("ones_col", [128, 1], dt.bfloat16)
    ones_row_f = win("ones_row_f", [1, 128])
    gamma_row = win("gamma_row", [1, 128])
    beta_row = win("beta_row", [1, 128])

    x_out = nc.dram_tensor("x_out", [NODES_PAD, HID], dt.float32, kind="ExternalOutput")
    r_out = nc.dram_tensor("r_out", [N_REL, HID], dt.float32, kind="ExternalOutput")

    xW = nc.dram_tensor("xW_tab", [X_PAD, HID], dt.bfloat16, kind="Internal")
    rWb = nc.dram_tensor("rWb_tab", [512, HID], dt.bfloat16, kind="Internal")

    with tile.TileContext(nc) as tc:
        with tc.tile_pool(name="cst", bufs=1) as cst, \
             tc.tile_pool(name="sb", bufs=3) as sb, \
             tc.tile_pool(name="ps", bufs=1, space="PSUM") as ps, \
             tc.tile_pool(name="psU", bufs=2, space="PSUM") as psU:

            # ---- constants resident in SBUF -----------------------------
            def ld_const(t, d=dt.float32, shape=(128, 128)):
                tl = cst.tile(list(shape), d, tag=t.name)
                nc.sync.dma_start(tl[:], t[:])
                return tl

            Wm1T_t = ld_const(Wm1T, dt.bfloat16)
            Wm2T_t = ld_const(Wm2T, dt.bfloat16)
            bmess_t = ld_const(bmess_bc)
            W1asT_t = ld_const(W1asT, dt.bfloat16)
            bias_as_t = ld_const(bias_as, dt.float32, (128, 1))
            spm_t = ld_const(spm_col, dt.bfloat16, (128, 1))
            u2_t = ld_const(u2s_col, dt.bfloat16, (128, 1))
            Wx1asT_t = ld_const(Wx1asT, dt.bfloat16)
            bias_xas_t = ld_const(bias_xas, dt.float32, (128, 1))
            spmx_t = ld_const(spmx_col, dt.bfloat16, (128, 1))
            ux_t = ld_const(ux_col, dt.bfloat16, (128, 1))
            iota_t = ld_const(iota_bc, dt.bfloat16)
            ones_t = ld_const(ones_col, dt.bfloat16, (128, 1))
            onesrow_t = ld_const(ones_row_f, dt.float32, (1, 128))
            onescf_t = ld_const(ones_colf, dt.float32, (128, 1))
            gamma_t = ld_const(gamma_row, dt.float32, (1, 128))
            beta_t = ld_const(beta_row, dt.float32, (1, 128))
            ident_t = cst.tile([128, 128], dt.float32, tag="ident")
            make_identity(nc, ident_t[:])
            ident_b = cst.tile([128, 128], dt.bfloat16, tag="identb")
            make_identity(nc, ident_b[:])
            headsT_t = cst.tile([128, NT], dt.int32, tag="headsT")
            nc.sync.dma_start(headsT_t[:], headsT[:])
            attrsT_t = cst.tile([128, NT], dt.int32, tag="attrsT")
            nc.sync.dma_start(attrsT_t[:], attrsT[:])
            tailsT_t = cst.tile([128, NT], dt.float32, tag="tailsT")
            nc.sync.dma_start(tailsT_t[:], tailsT[:])

            # ================= phase 1: BN(r) + rWb table ================
            # column sums of r and r^2 via ones-matmul accumulation
            stat_ps = ps.tile([1, 256], dt.float32, tag="acc", bufs=1)
            rsq = sb.tile([128, 128], dt.float32, tag="rsq")
            rchunks = []
            for i in range(4):
                rows = min(128, N_REL - i * 128)
                rt = sb.tile([128, 128], dt.float32, tag=f"rch{i}")
                nc.sync.dma_start(rt[:rows, :], r_in[i*128:i*128+rows, :])
                rchunks.append((rt, rows))
                nc.tensor.matmul(stat_ps[:1, 0:128], lhsT=onescf_t[:rows, :],
                                 rhs=rt[:rows, :], start=(i == 0), stop=(i == 3))
            # squares accumulated separately
            for i in range(4):
                rt, rows = rchunks[i]
                nc.scalar.square(rsq[:rows, :], rt[:rows, :])
                nc.tensor.matmul(stat_ps[:1, 128:256], lhsT=onescf_t[:rows, :],
                                 rhs=rsq[:rows, :], start=(i == 0), stop=(i == 3))
            mu = sb.tile([1, 128], dt.float32, tag="mu")
            nc.scalar.mul(mu[:], stat_ps[:1, 0:128], 1.0 / N_REL)
            var = sb.tile([1, 128], dt.float32, tag="var")
            nc.scalar.mul(var[:], stat_ps[:1, 128:256], 1.0 / N_REL)
            musq = sb.tile([1, 128], dt.float32, tag="musq")
            nc.vector.tensor_mul(musq[:], mu[:], mu[:])
            nc.vector.tensor_sub(var[:], var[:], musq[:])
            # scale = gamma * rsqrt(var + eps)
            nc.vector.tensor_scalar_add(var[:], var[:], BN_EPS)
            nc.scalar.activation(var[:], var[:], mybir.ActivationFunctionType.Sqrt)
            rinv = sb.tile([1, 128], dt.float32, tag="rinv")
            nc.vector.reciprocal(rinv[:], var[:])
            scale_r = sb.tile([1, 128], dt.float32, tag="scale_r")
            nc.vector.tensor_mul(scale_r[:], gamma_t[:], rinv[:])
            shift_r = sb.tile([1, 128], dt.float32, tag="shift_r")
            nc.vector.tensor_mul(shift_r[:], mu[:], scale_r[:])
            nc.vector.tensor_sub(shift_r[:], beta_t[:], shift_r[:])
            # broadcast rows to [128,128] via K=1 matmul
            bcast_ps = ps.tile([128, 256], dt.float32, tag="acc", bufs=1)
            nc.tensor.matmul(bcast_ps[:, 0:128], lhsT=onesrow_t[:], rhs=scale_r[:],
                             start=True, stop=True)
            nc.tensor.matmul(bcast_ps[:, 128:256], lhsT=onesrow_t[:], rhs=shift_r[:],
                             start=True, stop=True)
            scale_bc = sb.tile([128, 128], dt.float32, tag="scale_bc")
            nc.vector.tensor_copy(scale_bc[:], bcast_ps[:, 0:128])
            shift_bc = sb.tile([128, 128], dt.float32, tag="shift_bc")
            nc.vector.tensor_copy(shift_bc[:], bcast_ps[:, 128:256])

            for i in range(4):
                rt, rows = rchunks[i]
                rn = sb.tile([128, 128], dt.float32, tag="rn")
                nc.vector.tensor_mul(rn[:rows, :], rt[:rows, :], scale_bc[:rows, :])
                nc.vector.tensor_add(rn[:rows, :], rn[:rows, :], shift_bc[:rows, :])
                nc.sync.dma_start(r_out[i*128:i*128+rows, :], rn[:rows, :])
                # rWb chunk: transpose rn, matmul with Wm2T, add b_mess
                tp = ps.tile([128, 256], dt.float32, tag="trans", bufs=1)
                nc.tensor.transpose(tp[:, :rows], rn[:rows, :], ident_t[:rows, :rows])
                rnT = sb.tile([128, 128], dt.bfloat16, tag="rnT")
                nc.vector.tensor_copy(rnT[:, :rows], tp[:, :rows])
                mm = ps.tile([128, 256], dt.float32, tag="trans", bufs=1)
                nc.tensor.matmul(mm[:rows, :128], lhsT=rnT[:, :rows], rhs=Wm2T_t[:],
                                 start=True, stop=True)
                rwb = sb.tile([128, 128], dt.bfloat16, tag="rwb")
                nc.vector.tensor_add(rwb[:rows, :], mm[:rows, :128], bmess_t[:rows, :])
                nc.sync.dma_start(rWb[i*128:i*128+rows, :], rwb[:rows, :])
            zpad = sb.tile([12, 128], dt.bfloat16, tag="zpad")
            nc.vector.memset(zpad[:], 0.0)
            nc.sync.dma_start(rWb[500:512, :], zpad[:])

            # ================= phase 0: xW table ========================
            NB4 = X_CHUNKS // 4 + (1 if X_CHUNKS % 4 else 0)
            for b in range(NB4 if CUT != 1 else 0):
                c0 = b * 4
                nch = min(4, X_CHUNKS - c0)
                xT4 = sb.tile([128, 512], dt.bfloat16, tag="p0xT")
                nc.sync.dma_start(xT4[:, :nch*128],
                                  xT_full[:, c0*128:(c0+nch)*128])
                mmp = ps.tile([128, 512], dt.float32, tag="vps", bufs=1)
                for j in range(nch):
                    nc.tensor.matmul(mmp[:, j*128:(j+1)*128],
                                     lhsT=xT4[:, j*128:(j+1)*128],
                                     rhs=Wm1T_t[:], start=True, stop=True)
                ob = sb.tile([128, 512], dt.bfloat16, tag="p0ob")
                nc.any.tensor_copy(ob[:, :nch*128], mmp[:, :nch*128])
                nc.sync.dma_start(
                    xW[c0*128:(c0+nch)*128, :].rearrange("(j p) h -> p j h", p=128),
                    ob[:, :nch*128].rearrange("p (j h) -> p j h", h=128))

            # ================= phase 2: edges ===========================
            SUB = 7 if K >= 7 else K
            n_sub = (K + SUB - 1) // SUB
            for rep_i in range(REPEAT):
              for g in range(NG if CUT not in (1, 2) else 0):
                U = psU.tile([128, 132], dt.float32, tag="U")
                w_g = sb.tile([128, K], dt.float32, tag="w_g")
                mess_g = sb.tile([128, K, 132], dt.bfloat16, tag="mess_g")
                nc.vector.memset(mess_g[:, :, 128:129], 1.0)
                for si in range(n_sub):
                    k0 = si * SUB
                    k1 = min(K, k0 + SUB)
                    nk = k1 - k0
                    pre = sb.tile([128, SUB * 128], dt.bfloat16, tag="pre")
                    for kk in range(k0, k1):
                        t_idx = g * K + kk
                        j = kk - k0
                        nc.gpsimd.indirect_dma_start(
                            out=pre[:, j*128:(j+1)*128], out_offset=None,
                            in_=xW[:],
                            in_offset=bass.IndirectOffsetOnAxis(
                                ap=headsT_t[:, t_idx:t_idx+1], axis=0))
                        nc.gpsimd.indirect_dma_start(
                            out=pre[:, j*128:(j+1)*128], out_offset=None,
                            in_=rWb[:],
                            in_offset=bass.IndirectOffsetOnAxis(
                                ap=attrsT_t[:, t_idx:t_idx+1], axis=0),
                            compute_op=mybir.AluOpType.add)
                    if STAGE < 1:
                        continue
                    # tanh
                    mview = mess_g[:, k0:k0+nk, :128]
                    nc.scalar.activation(mview, pre[:, :nk*128].rearrange(
                        "p (k f) -> p k f", f=128),
                                         mybir.ActivationFunctionType.Tanh)
                    if STAGE < 2:
                        continue
                    # transpose each tile on PE
                    msT = sb.tile([128, SUB * 128], dt.bfloat16, tag="msT")
                    for j in range(nk):
                        tps = ps.tile([128, 128], dt.bfloat16, tag="tps", bufs=1)
                        nc.tensor.transpose(tps[:], mess_g[:, k0+j, :128],
                                            ident_b[:])
                        nc.any.tensor_copy(msT[:, j*128:(j+1)*128], tps[:])
                    # attention matmul [h' x nk*128]
                    vps = ps.tile([128, SUB * 128], dt.float32, tag="vps", bufs=1)
                    for mm0 in range(0, nk*128, 512):
                        mm1 = min(nk*128, mm0 + 512)
                        nc.tensor.matmul(vps[:, mm0:mm1], lhsT=W1asT_t[:],
                                         rhs=msT[:, mm0:mm1], start=True, stop=True)
                    relu = sb.tile([128, SUB * 128], dt.bfloat16, tag="relu")
                    nc.vector.tensor_scalar(
                        out=relu[:, :nk*128], in0=vps[:, :nk*128],
                        scalar1=bias_as_t[:], scalar2=0.0,
                        op0=mybir.AluOpType.add, op1=mybir.AluOpType.max)
                    if STAGE < 3:
                        continue
                    # attention logits: c = sum s*relu + (0.01/0.99)*mess.u2
                    cps = ps.tile([128, SUB], dt.float32, tag="cps", bufs=1)
                    for j in range(nk):
                        nc.tensor.matmul(cps[:, j:j+1],
                                         lhsT=relu[:, j*128:(j+1)*128],
                                         rhs=spm_t[:], start=True, stop=False)
                        nc.tensor.matmul(cps[:, j:j+1],
                                         lhsT=msT[:, j*128:(j+1)*128],
                                         rhs=u2_t[:], start=False, stop=True)
                    nc.scalar.activation(w_g[:, k0:k1], cps[:, :nk],
                                         mybir.ActivationFunctionType.Exp,
                                         scale=float(C["exp_scale"]))
                    if STAGE < 4:
                        continue
                    # scatter matmuls
                    for j in range(nk):
                        kk = k0 + j
                        t_idx = g * K + kk
                        sw = sb.tile([128, 128], dt.bfloat16, tag="sw")
                        nc.vector.tensor_scalar(
                            out=sw[:], in0=iota_t[:],
                            scalar1=tailsT_t[:, t_idx:t_idx+1],
                            scalar2=w_g[:, kk:kk+1],
                            op0=mybir.AluOpType.is_equal, op1=mybir.AluOpType.mult)
                        nc.tensor.matmul(U[:, 0:129], lhsT=sw[:],
                                         rhs=mess_g[:, kk, :129],
                                         start=(kk == 0), stop=(kk == K - 1))

                if STAGE < 5:
                    continue
                # ---- normalize + gating for this group -----------------
                zc = sb.tile([128, 1], dt.float32, tag="zc")
                nc.vector.tensor_scalar_add(zc[:], U[:, 128:129], 1e-16)
                rec = sb.tile([128, 1], dt.float32, tag="rec")
                nc.vector.reciprocal(rec[:], zc[:])
                sm_f = sb.tile([128, 128], dt.float32, tag="sm_f")
                nc.vector.tensor_scalar_mul(sm_f[:], U[:, 0:128], rec[:])
                sm_b = sb.tile([128, 128], dt.bfloat16, tag="sm_b")
                nc.vector.tensor_copy(sm_b[:], sm_f[:])
                xg = sb.tile([128, 128], dt.float32, tag="xg")
                nc.sync.dma_start(xg[:], x_slice[g*128:(g+1)*128, :])
                vT = sb.tile([128, 256], dt.bfloat16, tag="vT")
                nc.sync.dma_start(vT[:, 0:128], xT_slice[:, g*128:(g+1)*128])
                smtp = ps.tile([128, 128], dt.bfloat16, tag="tps", bufs=1)
                nc.tensor.transpose(smtp[:], sm_b[:], ident_b[:])
                nc.any.tensor_copy(vT[:, 128:256], smtp[:])
                gps = ps.tile([128, 256], dt.float32, tag="trans", bufs=1)
                nc.tensor.matmul(gps[:], lhsT=Wx1asT_t[:], rhs=vT[:],
                                 start=True, stop=True)
                grelu = sb.tile([128, 256], dt.bfloat16, tag="grelu")
                nc.vector.tensor_scalar(
                    out=grelu[:], in0=gps[:],
                    scalar1=bias_xas_t[:], scalar2=0.0,
                    op0=mybir.AluOpType.add, op1=mybir.AluOpType.max)
                # gating logits via PE dots
                gtp = ps.tile([128, 4], dt.float32, tag="cps", bufs=1)
                nc.tensor.matmul(gtp[:, 0:1], lhsT=grelu[:, 0:128],
                                 rhs=spmx_t[:], start=True, stop=True)
                nc.tensor.matmul(gtp[:, 1:2], lhsT=grelu[:, 128:256],
                                 rhs=spmx_t[:], start=True, stop=True)
                nc.tensor.matmul(gtp[:, 2:3], lhsT=vT[:, 0:128],
                                 rhs=ux_t[:], start=True, stop=True)
                nc.tensor.matmul(gtp[:, 3:4], lhsT=vT[:, 128:256],
                                 rhs=ux_t[:], start=True, stop=True)
                gt = sb.tile([128, 4], dt.float32, tag="gt")
                nc.any.tensor_copy(gt[:], gtp[:])
                # delta = (t2x0 - t2x1) + (t1x_x - t1x_sm)
                d01 = sb.tile([128, 2], dt.float32, tag="d01")
                nc.vector.tensor_sub(d01[:, 0:1], gt[:, 0:1], gt[:, 1:2])
                nc.vector.tensor_sub(d01[:, 1:2], gt[:, 2:3], gt[:, 3:4])
                w0 = sb.tile([128, 1], dt.float32, tag="w0")
                nc.vector.tensor_add(w0[:], d01[:, 0:1], d01[:, 1:2])
                nc.scalar.activation(w0[:], w0[:],
                                     mybir.ActivationFunctionType.Sigmoid,
                                     scale=float(C["sig_scale"]))
                # x_out = sm + w0 * (x - sm)
                dxm = sb.tile([128, 128], dt.float32, tag="dxm")
                nc.vector.tensor_sub(dxm[:], xg[:], sm_f[:])
                xo = sb.tile([128, 128], dt.float32, tag="xo")
                nc.vector.tensor_scalar_mul(xo[:], dxm[:], w0[:])
                nc.vector.tensor_add(xo[:], xo[:], sm_f[:])
                nc.sync.dma_start(x_out[g*128:(g+1)*128, :], xo[:])

        if CUT in (1, 2):
            with tc.tile_pool(name="zfill", bufs=1) as zf:
                zt = zf.tile([128, 128], dt.float32)
                nc.vector.memset(zt[:], 0.0)
                for g in range(NG):
                    nc.sync.dma_start(x_out[g*128:(g+1)*128, :], zt[:])

    nc.finalize()
    return nc


def kernel(**inputs):
    from concourse.bass_utils import run_bass_kernel_spmd

    in_maps, meta = _host_prep(**inputs)
    key = meta["K"]
    if key not in _COMPILED:
        _COMPILED[key] = _build(meta)
    nc = _COMPILED[key]
    res = run_bass_kernel_spmd(nc, in_maps, core_ids=list(range(N_CORES)))
    x_parts = [res.results[c]["x_out"][:NODES_PER_CORE] for c in range(N_CORES)]
    x_out = np.concatenate(x_parts, axis=0)
    r_out = res.results[0]["r_out"]
    return x_out, r_out


if __name__ == "__main__":
    import reference
    inputs = {k: np.asarray(v) for k, v in reference.setup_inputs().items()}
    got_x, got_r = kernel(**inputs)
    exp_x, exp_r = reference.reference(**reference.setup_inputs())
    exp_x = np.asarray(exp_x); exp_r = np.asarray(exp_r)
    for name, g, e in (("x_out", got_x, exp_x), ("r", got_r, exp_r)):
        rel = np.abs(g - e).max() / (np.abs(e).max() + 1e-9)
        print(f"{name}: max abs {np.abs(g-e).max():.5g}  rel {rel:.5g}")
